# revision 2
# baseline (speedup 1.0000x reference)
"""Trainium2 Bass kernel for PersonalizedCalibrationNetwork (MoE-style judge routing).

Strategy: expert-parallel over the judge axis. Judge j lives on core j // 8.
The host routes samples to the core owning their judge, groups them by judge,
and pads every judge group to a uniform capacity C (so the single SPMD program
is shape-identical on all 8 cores). Each core computes, for its samples:

    z1 = sigmoid(x_aug @ (W1 + W1a[j]))      x_aug = [x, 1]
    z2 = sigmoid(z1_aug @ (W2 + W2a[j]))
    out = z2_aug @ (V + Va[j])               flattened to [257, 64]

All matmuls run transposed (features on partitions): z^T = G^T @ x^T, so layer
outputs feed the next layer without transposes. The shared weight (W) part is
one full-width matmul per PSUM group; the judge-specific part accumulates into
the judge's column slice of the same PSUM bank; bias rows are K=1 matmuls
against a resident ones-row. Host pre-packs every tensor in the exact
[128-partition, free] SBUF layout so every DMA is contiguous.
"""

import numpy as np

import concourse.mybir as mybir
import concourse.tile as tile
from concourse import bacc
from concourse.bass_utils import run_bass_kernel_spmd

N_CORES = 8
J = 64                 # judges
JPC = J // N_CORES     # judges per core
IN = 256               # input features (+1 bias)
L1 = 256
L2 = 256
Q = 16
A = 4
QA = Q * A             # 64 output columns
P = 128                # partitions
PSUM_W = 512           # fp32 psum bank width

_cache = {}


def _build_program(C):
    """Build + compile the SPMD Bass program for per-judge capacity C."""
    N = JPC * C  # padded samples per core

    # judge groups sharing one PSUM bank: group width ≤ PSUM_W
    per_group = max(1, min(JPC, PSUM_W // C)) if C <= PSUM_W else 1
    groups = []
    if C <= PSUM_W:
        for g0 in range(0, JPC, per_group):
            js = list(range(g0, min(g0 + per_group, JPC)))
            # blocks: (judge, col offset in group, width, col offset in N)
            blocks = [(jj, (jj - g0) * C, C, jj * C) for jj in js]
            groups.append((g0 * C, len(js) * C, blocks))
    else:
        for jj in range(JPC):
            for c0 in range(0, C, PSUM_W):
                w = min(PSUM_W, C - c0)
                groups.append((jj * C + c0, w, [(jj, 0, w, jj * C + c0)]))

    nc = bacc.Bacc("TRN2", target_bir_lowering=False, debug=False,
                   num_devices=N_CORES)
    dt = mybir.dt.float32

    # DRAM inputs, pre-packed host-side in SBUF layout
    xT_d = nc.dram_tensor("xT", [P, 2, N], dt, kind="ExternalInput")
    w1_d = nc.dram_tensor("w1", [P, 2, L1], dt, kind="ExternalInput")
    w2_d = nc.dram_tensor("w2", [P, 2, L2], dt, kind="ExternalInput")
    v_d = nc.dram_tensor("v", [P, 2, QA], dt, kind="ExternalInput")
    w1a_d = nc.dram_tensor("w1a", [P, JPC, 2, L1], dt, kind="ExternalInput")
    w2a_d = nc.dram_tensor("w2a", [P, JPC, 2, L2], dt, kind="ExternalInput")
    va_d = nc.dram_tensor("va", [P, JPC, 2, QA], dt, kind="ExternalInput")
    # bias pack: [w1b(256) | w2b(256) | vb(64) | w1ab(8*256) | w2ab(8*256) | vab(8*64)]
    NB = L1 + L2 + QA + JPC * (L1 + L2 + QA)
    bias_d = nc.dram_tensor("biases", [1, NB], dt, kind="ExternalInput")
    out_d = nc.dram_tensor("outT", [QA, N], dt, kind="ExternalOutput")

    with tile.TileContext(nc) as tc:
        with (
            tc.tile_pool(name="const", bufs=1) as const,
            tc.tile_pool(name="psum", bufs=4, space="PSUM") as psum,
        ):
            xT = const.tile([P, 2, N], dt, tag="xT")
            w1 = const.tile([P, 2, L1], dt, tag="w1")
            biases = const.tile([1, NB], dt, tag="biases")
            ones = const.tile([1, max(N, 1)], dt, tag="ones")
            w1a = [const.tile([P, 2, L1], dt, tag=f"w1a{j}", name=f"w1a{j}") for j in range(JPC)]
            w2 = const.tile([P, 2, L2], dt, tag="w2")
            w2a = [const.tile([P, 2, L2], dt, tag=f"w2a{j}", name=f"w2a{j}") for j in range(JPC)]
            v = const.tile([P, 2, QA], dt, tag="v")
            va = [const.tile([P, 2, QA], dt, tag=f"va{j}", name=f"va{j}") for j in range(JPC)]
            z1T = const.tile([P, 2, N], dt, tag="z1T")
            z2T = const.tile([P, 2, N], dt, tag="z2T")
            outT = const.tile([QA, N], dt, tag="outT")

            # issue loads roughly in first-use order
            nc.sync.dma_start(xT[:], xT_d[:])
            nc.sync.dma_start(w1[:], w1_d[:])
            nc.sync.dma_start(biases[:], bias_d[:])
            nc.vector.memset(ones[:], 1.0)
            for j in range(JPC):
                nc.sync.dma_start(w1a[j][:], w1a_d[:, j])
            nc.sync.dma_start(w2[:], w2_d[:])
            for j in range(JPC):
                nc.sync.dma_start(w2a[j][:], w2a_d[:, j])
            nc.sync.dma_start(v[:], v_d[:])
            for j in range(JPC):
                nc.sync.dma_start(va[j][:], va_d[:, j])

            # bias slices within the pack
            def b_shared(layer):  # layer 0,1,2
                off = [0, L1, L1 + L2][layer]
                width = [L1, L2, QA][layer]
                return biases[:, off:off + width]

            def b_judge(layer, jj):
                base = L1 + L2 + QA
                if layer == 0:
                    off = base + jj * L1
                    width = L1
                elif layer == 1:
                    off = base + JPC * L1 + jj * L2
                    width = L2
                else:
                    off = base + JPC * (L1 + L2) + jj * QA
                    width = QA
                return biases[:, off:off + width]

            def layer(w_sh, w_jd, rhs, bias_layer, M, zout):
                """z^T[M, N] = act(W^T @ rhs), accumulated per group."""
                n_m = (M + P - 1) // P
                for col0, gw, blocks in groups:
                    for m in range(n_m):
                        mw = min(P, M - m * P)
                        ps = psum.tile([P, PSUM_W], dt, tag="ps", name="ps")[:mw, :gw]
                        ms = slice(m * P, m * P + mw)
                        for ko in range(2):
                            nc.tensor.matmul(
                                ps, w_sh[:, ko, ms],
                                rhs[:, ko, col0:col0 + gw],
                                start=(ko == 0), stop=False)
                        nc.tensor.matmul(
                            ps, b_shared(bias_layer)[:, ms],
                            ones[:, :gw], start=False, stop=False)
                        for bi, (jj, off, w, ncol) in enumerate(blocks):
                            for ko in range(2):
                                nc.tensor.matmul(
                                    ps[:, off:off + w], w_jd[jj][:, ko, ms],
                                    rhs[:, ko, ncol:ncol + w],
                                    start=False, stop=False)
                            nc.tensor.matmul(
                                ps[:, off:off + w], b_judge(bias_layer, jj)[:, ms],
                                ones[:, :w], start=False,
                                stop=(bi == len(blocks) - 1))
                        if zout is not None:
                            nc.scalar.activation(
                                zout[:mw, m, col0:col0 + gw], ps,
                                mybir.ActivationFunctionType.Sigmoid)
                        else:
                            nc.vector.tensor_copy(
                                outT[:mw, col0:col0 + gw], ps)

            layer(w1, w1a, xT, 0, L1, z1T)
            layer(w2, w2a, z1T, 1, L2, z2T)
            layer(v, va, z2T, 2, QA, None)

            nc.sync.dma_start(out_d[:], outT[:])

    nc.compile()
    return nc, N


def kernel(X_machine_evals, X_human_judges, W1, W1a, W2, W2a, V, Va):
    X = np.ascontiguousarray(np.asarray(X_machine_evals, dtype=np.float32))
    jid = np.asarray(X_human_judges).reshape(-1).astype(np.int64)
    W1 = np.asarray(W1, dtype=np.float32)
    W1a = np.asarray(W1a, dtype=np.float32)
    W2 = np.asarray(W2, dtype=np.float32)
    W2a = np.asarray(W2a, dtype=np.float32)
    V = np.asarray(V, dtype=np.float32)
    Va = np.asarray(Va, dtype=np.float32)
    B = X.shape[0]

    counts = np.bincount(jid, minlength=J)
    C = int(counts.max())

    if C not in _cache:
        _cache[C] = _build_program(C)
    nc, N = _cache[C]

    # stable order of sample indices grouped by judge
    order = np.argsort(jid, kind="stable")
    sorted_j = jid[order]

    def pack_w(w):  # [256, M] -> [128, 2, M]
        M = w.shape[1]
        return np.ascontiguousarray(
            w[:256].reshape(2, P, M).transpose(1, 0, 2))

    Vf = V.transpose(1, 0, 2).reshape(IN + 1, QA)        # [257, 64]
    Vaf = Va.transpose(0, 2, 1, 3).reshape(J, IN + 1, QA)  # [J, 257, 64]

    w1_in = pack_w(W1)
    w2_in = pack_w(W2)
    v_in = pack_w(Vf)

    in_maps = []
    core_meta = []
    for c in range(N_CORES):
        judges = np.arange(c * JPC, (c + 1) * JPC)
        Xp = np.zeros((N, IN), dtype=np.float32)
        samp = []  # (column, sample index)
        for k, jj in enumerate(judges):
            idx = order[np.searchsorted(sorted_j, jj):
                        np.searchsorted(sorted_j, jj, side="right")]
            Xp[k * C:k * C + len(idx)] = X[idx]
            samp.append(idx)
        core_meta.append(samp)

        xT_in = np.ascontiguousarray(
            Xp.T.reshape(2, P, N).transpose(1, 0, 2))
        w1a_in = np.stack([pack_w(W1a[jj]) for jj in judges], axis=1)
        w2a_in = np.stack([pack_w(W2a[jj]) for jj in judges], axis=1)
        va_in = np.stack([pack_w(Vaf[jj]) for jj in judges], axis=1)
        bias_in = np.concatenate([
            W1[256], W2[256], Vf[256],
            W1a[judges, 256].reshape(-1),
            W2a[judges, 256].reshape(-1),
            Vaf[judges, 256].reshape(-1),
        ]).astype(np.float32)[None, :]
        in_maps.append({
            "xT": xT_in, "w1": w1_in, "w2": w2_in, "v": v_in,
            "w1a": np.ascontiguousarray(w1a_in),
            "w2a": np.ascontiguousarray(w2a_in),
            "va": np.ascontiguousarray(va_in),
            "biases": np.ascontiguousarray(bias_in),
        })

    res = run_bass_kernel_spmd(nc, in_maps, core_ids=list(range(N_CORES)))

    out = np.zeros((B, Q, A), dtype=np.float32)
    for c in range(N_CORES):
        oT = res.results[c]["outT"]          # [64, N]
        o = oT.T.reshape(N, Q, A)
        for k, idx in enumerate(core_meta[c]):
            out[idx] = o[k * C:k * C + len(idx)]
    return out


# revision 3
# speedup vs baseline: 2.6230x; 2.6230x over previous
"""Trainium2 Bass kernel for PersonalizedCalibrationNetwork (MoE-style judge routing).

Strategy: expert-parallel over the judge axis. Judge j lives on core j // 8.
The host routes samples to the core owning their judge, groups them by judge,
and pads every judge group to a uniform capacity C (so the single SPMD program
is shape-identical on all 8 cores). Each core computes, for its samples:

    z1 = sigmoid(x_aug @ (W1 + W1a[j]))      x_aug = [x, 1]
    z2 = sigmoid(z1_aug @ (W2 + W2a[j]))
    out = z2_aug @ (V + Va[j])               flattened to [257, 64]

All matmuls run transposed (features on partitions): z^T = G^T @ x^T, so layer
outputs feed the next layer without transposes. Per PSUM group (a bank-wide
strip of judge column-blocks):
  - the shared weight part is a full-width matmul,
  - each judge's weight part accumulates into its column slice,
  - all bias rows (8 judge + 1 shared) are applied by ONE K=9 matmul against a
    host-built 0/1 block mask [9, N].
Inputs are bf16 (fp32 accumulation in PSUM); host pre-packs every tensor in the
exact [128-partition, free] SBUF layout so every DMA is contiguous.
"""

import ml_dtypes
import numpy as np

import concourse.mybir as mybir
import concourse.tile as tile
from concourse import bacc
from concourse.bass_utils import run_bass_kernel_spmd

N_CORES = 8
J = 64                 # judges
JPC = J // N_CORES     # judges per core
IN = 256               # input features (+1 bias)
L1 = 256
L2 = 256
Q = 16
A = 4
QA = Q * A             # 64 output columns
P = 128                # partitions
PSUM_W = 512           # fp32 psum bank width
KB = JPC + 1           # bias-matmul contraction dim (8 judge rows + shared)

BF16 = True
NP_W = ml_dtypes.bfloat16 if BF16 else np.float32

_cache = {}


def _build_program(C):
    """Build + compile the SPMD Bass program for per-judge capacity C."""
    N = JPC * C  # padded samples per core

    # judge groups sharing one PSUM bank: group width ≤ PSUM_W
    groups = []
    if C <= PSUM_W:
        per_group = max(1, min(JPC, PSUM_W // C))
        for g0 in range(0, JPC, per_group):
            js = list(range(g0, min(g0 + per_group, JPC)))
            # blocks: (judge, col offset in N, width)
            blocks = [(jj, jj * C, C) for jj in js]
            groups.append((g0 * C, len(js) * C, blocks))
    else:
        for jj in range(JPC):
            for c0 in range(0, C, PSUM_W):
                w = min(PSUM_W, C - c0)
                groups.append((jj * C + c0, w, [(jj, jj * C + c0, w)]))

    nc = bacc.Bacc("TRN2", target_bir_lowering=False, debug=False,
                   num_devices=N_CORES)
    dt = mybir.dt.bfloat16 if BF16 else mybir.dt.float32
    f32 = mybir.dt.float32

    # DRAM inputs, pre-packed host-side in SBUF layout
    xT_d = nc.dram_tensor("xT", [P, 2, N], dt, kind="ExternalInput")
    w1_d = nc.dram_tensor("w1", [P, 2, L1], dt, kind="ExternalInput")
    w2_d = nc.dram_tensor("w2", [P, 2, L2], dt, kind="ExternalInput")
    v_d = nc.dram_tensor("v", [P, 2, QA], dt, kind="ExternalInput")
    w1a_d = nc.dram_tensor("w1a", [P, JPC, 2, L1], dt, kind="ExternalInput")
    w2a_d = nc.dram_tensor("w2a", [P, JPC, 2, L2], dt, kind="ExternalInput")
    va_d = nc.dram_tensor("va", [P, JPC, 2, QA], dt, kind="ExternalInput")
    # bias rows [KB, 576]: cols [0:256) L1, [256:512) L2, [512:576) out;
    # rows 0..7 = per-judge bias, row 8 = shared bias
    NB = L1 + L2 + QA
    bias_d = nc.dram_tensor("biases", [KB, NB], dt, kind="ExternalInput")
    # block mask [KB, N]: row jj = 1 on judge jj's columns, row 8 = all ones
    mask_d = nc.dram_tensor("mask", [KB, N], dt, kind="ExternalInput")
    out_d = nc.dram_tensor("outT", [QA, N], f32, kind="ExternalOutput")

    with tile.TileContext(nc) as tc:
        with (
            tc.tile_pool(name="const", bufs=1) as const,
            tc.tile_pool(name="psum", bufs=4, space="PSUM") as psum,
        ):
            xT = const.tile([P, 2, N], dt, tag="xT")
            w1 = const.tile([P, 2, L1], dt, tag="w1")
            biases = const.tile([KB, NB], dt, tag="biases")
            mask = const.tile([KB, N], dt, tag="mask")
            w1a = [const.tile([P, 2, L1], dt, tag=f"w1a{j}", name=f"w1a{j}")
                   for j in range(JPC)]
            w2 = const.tile([P, 2, L2], dt, tag="w2")
            w2a = [const.tile([P, 2, L2], dt, tag=f"w2a{j}", name=f"w2a{j}")
                   for j in range(JPC)]
            v = const.tile([P, 2, QA], dt, tag="v")
            va = [const.tile([P, 2, QA], dt, tag=f"va{j}", name=f"va{j}")
                  for j in range(JPC)]
            z1T = const.tile([P, 2, N], dt, tag="z1T")
            z2T = const.tile([P, 2, N], dt, tag="z2T")
            outT = const.tile([QA, N], f32, tag="outT")

            # issue loads roughly in first-use order
            nc.sync.dma_start(xT[:], xT_d[:])
            nc.sync.dma_start(w1[:], w1_d[:])
            nc.sync.dma_start(biases[:], bias_d[:])
            nc.sync.dma_start(mask[:], mask_d[:])
            for j in range(JPC):
                nc.sync.dma_start(w1a[j][:], w1a_d[:, j])
            nc.sync.dma_start(w2[:], w2_d[:])
            for j in range(JPC):
                nc.sync.dma_start(w2a[j][:], w2a_d[:, j])
            nc.sync.dma_start(v[:], v_d[:])
            for j in range(JPC):
                nc.sync.dma_start(va[j][:], va_d[:, j])

            def layer(w_sh, w_jd, rhs, b_off, M, zout):
                """z^T[M, N] = act(W^T @ rhs + b), accumulated per group."""
                n_m = (M + P - 1) // P
                for col0, gw, blocks in groups:
                    for m in range(n_m):
                        mw = min(P, M - m * P)
                        ps = psum.tile([P, PSUM_W], f32, tag="ps",
                                       name="ps")[:mw, :gw]
                        ms = slice(m * P, m * P + mw)
                        for ko in range(2):
                            nc.tensor.matmul(
                                ps, w_sh[:, ko, ms],
                                rhs[:, ko, col0:col0 + gw],
                                start=(ko == 0), stop=False)
                        nc.tensor.matmul(
                            ps, biases[:, b_off + m * P:b_off + m * P + mw],
                            mask[:, col0:col0 + gw], start=False, stop=False)
                        for bi, (jj, ncol, w) in enumerate(blocks):
                            off = ncol - col0
                            for ko in range(2):
                                nc.tensor.matmul(
                                    ps[:, off:off + w], w_jd[jj][:, ko, ms],
                                    rhs[:, ko, ncol:ncol + w],
                                    start=False,
                                    stop=(bi == len(blocks) - 1 and ko == 1))
                        if zout is not None:
                            nc.scalar.activation(
                                zout[:mw, m, col0:col0 + gw], ps,
                                mybir.ActivationFunctionType.Sigmoid)
                        else:
                            nc.vector.tensor_copy(
                                outT[:mw, col0:col0 + gw], ps)

            layer(w1, w1a, xT, 0, L1, z1T)
            layer(w2, w2a, z1T, L1, L2, z2T)
            layer(v, va, z2T, L1 + L2, QA, None)

            nc.sync.dma_start(out_d[:], outT[:])

    nc.compile()
    return nc, N


def kernel(X_machine_evals, X_human_judges, W1, W1a, W2, W2a, V, Va):
    X = np.asarray(X_machine_evals, dtype=np.float32)
    jid = np.asarray(X_human_judges).reshape(-1).astype(np.int64)
    W1 = np.asarray(W1, dtype=np.float32)
    W1a = np.asarray(W1a, dtype=np.float32)
    W2 = np.asarray(W2, dtype=np.float32)
    W2a = np.asarray(W2a, dtype=np.float32)
    V = np.asarray(V, dtype=np.float32)
    Va = np.asarray(Va, dtype=np.float32)
    B = X.shape[0]

    counts = np.bincount(jid, minlength=J)
    C = int(counts.max())

    if C not in _cache:
        _cache[C] = _build_program(C)
    nc, N = _cache[C]

    # stable order of sample indices grouped by judge
    order = np.argsort(jid, kind="stable")
    sorted_j = jid[order]

    def pack_w(w):  # [256, M] -> [128, 2, M]
        M = w.shape[1]
        return np.ascontiguousarray(
            w[:256].reshape(2, P, M).transpose(1, 0, 2).astype(NP_W))

    Vf = V.transpose(1, 0, 2).reshape(IN + 1, QA)          # [257, 64]
    Vaf = Va.transpose(0, 2, 1, 3).reshape(J, IN + 1, QA)  # [J, 257, 64]

    w1_in = pack_w(W1)
    w2_in = pack_w(W2)
    v_in = pack_w(Vf)

    mask_in = np.zeros((KB, N), dtype=NP_W)
    mask_in[JPC, :] = 1
    for k in range(JPC):
        mask_in[k, k * C:(k + 1) * C] = 1

    in_maps = []
    core_meta = []
    for c in range(N_CORES):
        judges = np.arange(c * JPC, (c + 1) * JPC)
        Xp = np.zeros((N, IN), dtype=np.float32)
        samp = []  # per-judge sample indices
        for k, jj in enumerate(judges):
            idx = order[np.searchsorted(sorted_j, jj):
                        np.searchsorted(sorted_j, jj, side="right")]
            Xp[k * C:k * C + len(idx)] = X[idx]
            samp.append(idx)
        core_meta.append(samp)

        xT_in = np.ascontiguousarray(
            Xp.T.reshape(2, P, N).transpose(1, 0, 2).astype(NP_W))
        w1a_in = np.ascontiguousarray(
            np.stack([pack_w(W1a[jj]) for jj in judges], axis=1))
        w2a_in = np.ascontiguousarray(
            np.stack([pack_w(W2a[jj]) for jj in judges], axis=1))
        va_in = np.ascontiguousarray(
            np.stack([pack_w(Vaf[jj]) for jj in judges], axis=1))
        bias_in = np.empty((KB, L1 + L2 + QA), dtype=np.float32)
        bias_in[:JPC, :L1] = W1a[judges, 256]
        bias_in[:JPC, L1:L1 + L2] = W2a[judges, 256]
        bias_in[:JPC, L1 + L2:] = Vaf[judges, 256]
        bias_in[JPC] = np.concatenate([W1[256], W2[256], Vf[256]])
        in_maps.append({
            "xT": xT_in, "w1": w1_in, "w2": w2_in, "v": v_in,
            "w1a": w1a_in, "w2a": w2a_in, "va": va_in,
            "biases": bias_in.astype(NP_W),
            "mask": mask_in,
        })

    res = run_bass_kernel_spmd(nc, in_maps, core_ids=list(range(N_CORES)))

    out = np.zeros((B, Q, A), dtype=np.float32)
    for c in range(N_CORES):
        oT = res.results[c]["outT"]          # [64, N]
        o = oT.T.reshape(N, Q, A)
        for k, idx in enumerate(core_meta[c]):
            out[idx] = o[k * C:k * C + len(idx)]
    return out


# revision 5
# speedup vs baseline: 3.2448x; 1.2371x over previous
"""Trainium2 Bass kernel for PersonalizedCalibrationNetwork (MoE-style judge routing).

Strategy: expert-parallel over the judge axis. Judge j lives on core j // 8.
The host routes samples to the core owning their judge, groups them by judge,
and pads every judge group to a uniform capacity C (so the single SPMD program
is shape-identical on all 8 cores). Each core computes, for its samples:

    z1 = sigmoid(x_aug @ (W1 + W1a[j]))      x_aug = [x, 1]
    z2 = sigmoid(z1_aug @ (W2 + W2a[j]))
    out = z2_aug @ (V + Va[j])               flattened to [257, 64]

All matmuls run transposed (features on partitions): z^T = G^T @ x^T, so layer
outputs feed the next layer without transposes. Per PSUM group (a bank-wide
strip of judge column-blocks):
  - the shared weight part is a full-width matmul,
  - each judge's weight part accumulates into its column slice,
  - all bias rows (8 judge + 1 shared) are applied by ONE K=9 matmul against a
    host-built 0/1 block mask [9, N].
Inputs are bf16 (fp32 accumulation in PSUM); host pre-packs every tensor in the
exact [128-partition, free] SBUF layout so every DMA is a single contiguous
transfer, and the 7 transfers are spread over 3 DGE sequencers (sync, scalar,
gpsimd) to avoid issue serialization.
"""

import ml_dtypes
import numpy as np

import concourse.mybir as mybir
import concourse.tile as tile
from concourse import bacc
from concourse.bass_utils import run_bass_kernel_spmd

N_CORES = 8
J = 64                 # judges
JPC = J // N_CORES     # judges per core
IN = 256               # input features (+1 bias)
L1 = 256
L2 = 256
Q = 16
A = 4
QA = Q * A             # 64 output columns
P = 128                # partitions
PSUM_W = 512           # fp32 psum bank width
KB = JPC + 1           # bias-matmul contraction dim (8 judge rows + shared)
NB = L1 + L2 + QA      # bias pack columns

BF16 = True
NP_W = ml_dtypes.bfloat16 if BF16 else np.float32

_cache = {}


def _build_program(C):
    """Build + compile the SPMD Bass program for per-judge capacity C."""
    N = JPC * C  # padded samples per core

    # judge groups sharing one PSUM bank: group width ≤ PSUM_W
    groups = []
    if C <= PSUM_W:
        per_group = max(1, min(JPC, PSUM_W // C))
        for g0 in range(0, JPC, per_group):
            js = list(range(g0, min(g0 + per_group, JPC)))
            # blocks: (judge, col offset in N, width)
            blocks = [(jj, jj * C, C) for jj in js]
            groups.append((g0 * C, len(js) * C, blocks))
    else:
        for jj in range(JPC):
            for c0 in range(0, C, PSUM_W):
                w = min(PSUM_W, C - c0)
                groups.append((jj * C + c0, w, [(jj, jj * C + c0, w)]))

    nc = bacc.Bacc("TRN2", target_bir_lowering=False, debug=False,
                   num_devices=N_CORES)
    dt = mybir.dt.bfloat16 if BF16 else mybir.dt.float32
    f32 = mybir.dt.float32

    # DRAM inputs, pre-packed host-side in SBUF layout
    xT_d = nc.dram_tensor("xT", [P, 2, N], dt, kind="ExternalInput")
    # shared weights [P, 2, 576]: cols [0:256) W1, [256:512) W2, [512:576) Vf
    wsh_d = nc.dram_tensor("wsh", [P, 2, NB], dt, kind="ExternalInput")
    w1a_d = nc.dram_tensor("w1a", [P, JPC, 2, L1], dt, kind="ExternalInput")
    w2a_d = nc.dram_tensor("w2a", [P, JPC, 2, L2], dt, kind="ExternalInput")
    va_d = nc.dram_tensor("va", [P, JPC, 2, QA], dt, kind="ExternalInput")
    # bias rows + block mask combined [KB, NB + N]:
    #   cols [0:NB) bias (rows 0..7 judge, row 8 shared),
    #   cols [NB:NB+N) mask (row jj = 1 on judge jj's columns, row 8 = ones)
    bm_d = nc.dram_tensor("bm", [KB, NB + N], dt, kind="ExternalInput")
    out_d = nc.dram_tensor("outT", [QA, N], f32, kind="ExternalOutput")

    with tile.TileContext(nc) as tc:
        with (
            tc.tile_pool(name="const", bufs=1) as const,
            tc.tile_pool(name="psum", bufs=6, space="PSUM") as psum,
        ):
            xT = const.tile([P, 2, N], dt, tag="xT")
            wsh = const.tile([P, 2, NB], dt, tag="wsh")
            bm = const.tile([KB, NB + N], dt, tag="bm")
            w1a = const.tile([P, JPC, 2, L1], dt, tag="w1a")
            w2a = const.tile([P, JPC, 2, L2], dt, tag="w2a")
            va = const.tile([P, JPC, 2, QA], dt, tag="va")
            z1T = const.tile([P, 2, N], dt, tag="z1T")
            z2T = const.tile([P, 2, N], dt, tag="z2T")
            outT = const.tile([QA, N], f32, tag="outT")

            # spread the loads over 3 DGE sequencers, first-use order
            nc.sync.dma_start(xT[:], xT_d[:])
            nc.scalar.dma_start(wsh[:], wsh_d[:])
            nc.gpsimd.dma_start(bm[:], bm_d[:])
            nc.sync.dma_start(w1a[:], w1a_d[:])
            nc.scalar.dma_start(w2a[:], w2a_d[:])
            nc.gpsimd.dma_start(va[:], va_d[:])

            def layer(sh_off, w_jd, rhs, M, zout):
                """z^T[M, N] = act(W^T @ rhs + b), accumulated per group."""
                n_m = (M + P - 1) // P
                for col0, gw, blocks in groups:
                    for m in range(n_m):
                        mw = min(P, M - m * P)
                        ps = psum.tile([P, PSUM_W], f32, tag="ps",
                                       name="ps")[:mw, :gw]
                        ms = slice(sh_off + m * P, sh_off + m * P + mw)
                        for ko in range(2):
                            nc.tensor.matmul(
                                ps, wsh[:, ko, ms],
                                rhs[:, ko, col0:col0 + gw],
                                start=(ko == 0), stop=False)
                        nc.tensor.matmul(
                            ps, bm[:, ms],
                            bm[:, NB + col0:NB + col0 + gw],
                            start=False, stop=False)
                        for bi, (jj, ncol, w) in enumerate(blocks):
                            off = ncol - col0
                            for ko in range(2):
                                nc.tensor.matmul(
                                    ps[:, off:off + w],
                                    w_jd[:, jj, ko, m * P:m * P + mw],
                                    rhs[:, ko, ncol:ncol + w],
                                    start=False,
                                    stop=(bi == len(blocks) - 1 and ko == 1))
                        if zout is not None:
                            nc.scalar.activation(
                                zout[:mw, m, col0:col0 + gw], ps,
                                mybir.ActivationFunctionType.Sigmoid)
                        else:
                            nc.vector.tensor_copy(
                                outT[:mw, col0:col0 + gw], ps)

            layer(0, w1a, xT, L1, z1T)
            layer(L1, w2a, z1T, L2, z2T)
            layer(L1 + L2, va, z2T, QA, None)

            nc.sync.dma_start(out_d[:], outT[:])

    nc.compile()
    return nc, N


def kernel(X_machine_evals, X_human_judges, W1, W1a, W2, W2a, V, Va):
    X = np.asarray(X_machine_evals, dtype=np.float32)
    jid = np.asarray(X_human_judges).reshape(-1).astype(np.int64)
    W1 = np.asarray(W1, dtype=np.float32)
    W1a = np.asarray(W1a, dtype=np.float32)
    W2 = np.asarray(W2, dtype=np.float32)
    W2a = np.asarray(W2a, dtype=np.float32)
    V = np.asarray(V, dtype=np.float32)
    Va = np.asarray(Va, dtype=np.float32)
    B = X.shape[0]

    counts = np.bincount(jid, minlength=J)
    C = int(counts.max())

    if C not in _cache:
        _cache[C] = _build_program(C)
    nc, N = _cache[C]

    # stable order of sample indices grouped by judge
    order = np.argsort(jid, kind="stable")
    sorted_j = jid[order]

    def pack_w(w):  # [256, M] -> [128, 2, M]
        M = w.shape[1]
        return np.ascontiguousarray(
            w[:256].reshape(2, P, M).transpose(1, 0, 2).astype(NP_W))

    Vf = V.transpose(1, 0, 2).reshape(IN + 1, QA)          # [257, 64]
    Vaf = Va.transpose(0, 2, 1, 3).reshape(J, IN + 1, QA)  # [J, 257, 64]

    wsh_in = np.concatenate([pack_w(W1), pack_w(W2), pack_w(Vf)], axis=2)
    wsh_in = np.ascontiguousarray(wsh_in)

    mask_in = np.zeros((KB, N), dtype=np.float32)
    mask_in[JPC, :] = 1
    for k in range(JPC):
        mask_in[k, k * C:(k + 1) * C] = 1

    in_maps = []
    core_meta = []
    for c in range(N_CORES):
        judges = np.arange(c * JPC, (c + 1) * JPC)
        Xp = np.zeros((N, IN), dtype=np.float32)
        samp = []  # per-judge sample indices
        for k, jj in enumerate(judges):
            idx = order[np.searchsorted(sorted_j, jj):
                        np.searchsorted(sorted_j, jj, side="right")]
            Xp[k * C:k * C + len(idx)] = X[idx]
            samp.append(idx)
        core_meta.append(samp)

        xT_in = np.ascontiguousarray(
            Xp.T.reshape(2, P, N).transpose(1, 0, 2).astype(NP_W))
        w1a_in = np.ascontiguousarray(
            np.stack([pack_w(W1a[jj]) for jj in judges], axis=1))
        w2a_in = np.ascontiguousarray(
            np.stack([pack_w(W2a[jj]) for jj in judges], axis=1))
        va_in = np.ascontiguousarray(
            np.stack([pack_w(Vaf[jj]) for jj in judges], axis=1))
        bm_in = np.empty((KB, NB + N), dtype=np.float32)
        bm_in[:JPC, :L1] = W1a[judges, 256]
        bm_in[:JPC, L1:L1 + L2] = W2a[judges, 256]
        bm_in[:JPC, L1 + L2:NB] = Vaf[judges, 256]
        bm_in[JPC, :NB] = np.concatenate([W1[256], W2[256], Vf[256]])
        bm_in[:, NB:] = mask_in
        in_maps.append({
            "xT": xT_in, "wsh": wsh_in,
            "w1a": w1a_in, "w2a": w2a_in, "va": va_in,
            "bm": bm_in.astype(NP_W),
        })

    res = run_bass_kernel_spmd(nc, in_maps, core_ids=list(range(N_CORES)))

    out = np.zeros((B, Q, A), dtype=np.float32)
    for c in range(N_CORES):
        oT = res.results[c]["outT"]          # [64, N]
        o = oT.T.reshape(N, Q, A)
        for k, idx in enumerate(core_meta[c]):
            out[idx] = o[k * C:k * C + len(idx)]
    return out
